# revision 12
# baseline (speedup 1.0000x reference)
"""Trainium2 Bass kernel for MHA cross-attention (nn_MHACross).

Sharding: 8 cores = 2 batches x 4 head-groups (2 heads each).
Each core computes, for its (batch b, head group g):
    q = x[b] @ Wq[g].T ; k,v = xmel[b] @ Wkv[g].T ; RoPE(q, k) (scale folded
    into cos/sin tables); scores^T = k_r @ q_r^T per head; p = exp(scores)
    (no max subtraction -- scores are O(6) here, safe in fp32); unnormalized
    out2 = v^T @ p and Z = ones^T @ p via PE; normalize; y_partial = attn @
    Wout[:, g].T.  Host sums the 4 partial y per batch.

All matmuls use float32r (TF32-like: full speed at N>=256, ~1.5e-4 rel err).
Layouts keep the contraction dim on partitions everywhere, so there are no
on-device transposes:
    xT/xmelT      [C, T/S]   (pre-transposed on host)
    qT_r, kT_r    [D=128, T/S]  per head
    v             [S-tile=128, 2*D]  (both heads side by side, N=256 matmuls)
    scores^T      [S-tile, T-chunk=512]
    out2          [D, T-chunk] accumulated over S-tiles in PSUM
    Z             [1, T-chunk] accumulated via ones-matmul
    y             [T-tile=128, C]
"""
import sys
sys.path.insert(0, '/opt/trn_rl_repo')
import numpy as np

DIM = 1024
NHEADS = 8
HD = 128          # head dim
HPC = 2           # heads per core
NG = 4            # head groups (cores per batch)
B, T, S = 2, 2048, 3000
NKT = DIM // 128  # contraction tiles
ROPE_BASE = 10000.0
SWAP_MASK = [(i + 16) % 32 for i in range(32)]

_cache = {}


def _ceil_div(a, b):
    return (a + b - 1) // b


def build_nc(T=T, S=S):
    from concourse import bacc, mybir
    from concourse.tile import TileContext

    f32 = mybir.dt.float32
    f32r = mybir.dt.float32r

    nc = bacc.Bacc("TRN2", target_bir_lowering=False, debug=False, num_devices=8)

    xT = nc.dram_tensor("xT", [DIM, T], f32, kind="ExternalInput")
    xmelT = nc.dram_tensor("xmelT", [DIM, S], f32, kind="ExternalInput")
    WqT = nc.dram_tensor("WqT", [DIM, HPC * HD], f32, kind="ExternalInput")
    WkT = nc.dram_tensor("WkT", [DIM, HPC * HD], f32, kind="ExternalInput")
    WvT = nc.dram_tensor("WvT", [DIM, HPC * HD], f32, kind="ExternalInput")
    WoT = nc.dram_tensor("WoT", [HPC * HD, DIM], f32, kind="ExternalInput")
    cosq = nc.dram_tensor("cosq", [HD, T], f32, kind="ExternalInput")
    sinq = nc.dram_tensor("sinq", [HD, T], f32, kind="ExternalInput")
    cosk = nc.dram_tensor("cosk", [HD, S], f32, kind="ExternalInput")
    sink = nc.dram_tensor("sink", [HD, S], f32, kind="ExternalInput")
    y = nc.dram_tensor("y", [T, DIM], f32, kind="ExternalOutput")
    # DRAM scratch for the Z-broadcast bounce (internal DRAM tensors fail to
    # load under the axon PJRT path, so declare it as an output)
    n_tc = _ceil_div(T, 512)
    zsd = nc.dram_tensor("zs", [HPC * n_tc, 512], f32, kind="ExternalOutput")

    n_st = _ceil_div(S, 128)                      # S-tiles (partition dim of scores^T)
    s_chunks = [(i * 512, min(512, S - i * 512)) for i in range(_ceil_div(S, 512))]
    t_chunks = [(i * 512, min(512, T - i * 512)) for i in range(_ceil_div(T, 512))]

    with TileContext(nc) as tc:
        with tc.tile_pool(name="wpool", bufs=1) as wp, \
             tc.tile_pool(name="persist", bufs=1) as pp:
            # persistent weights
            wq = wp.tile([128, NKT, HPC * HD], f32r)
            wk = wp.tile([128, NKT, HPC * HD], f32r)
            wv = wp.tile([128, NKT, HPC * HD], f32r)
            nc.sync.dma_start(out=wq[:], in_=WqT[:].rearrange("(k p) n -> p k n", p=128).bitcast(f32r))
            nc.sync.dma_start(out=wk[:], in_=WkT[:].rearrange("(k p) n -> p k n", p=128).bitcast(f32r))
            nc.sync.dma_start(out=wv[:], in_=WvT[:].rearrange("(k p) n -> p k n", p=128).bitcast(f32r))
            wo = []
            for h in range(HPC):
                wo_h = wp.tile([128, DIM], f32r, name=f"wo{h}", uniquify=True)
                nc.sync.dma_start(out=wo_h[:], in_=WoT[h * HD:(h + 1) * HD, :].bitcast(f32r))
                wo.append(wo_h)
            ones_f32 = wp.tile([128, 1], f32)
            nc.vector.memset(ones_f32[:], 1.0)
            ones = wp.tile([128, 1], f32r)
            nc.vector.tensor_copy(ones[:], ones_f32[:])

            # persistent activations
            kT_r = [pp.tile([128, S], f32r, name=f"kT{h}", uniquify=True) for h in range(HPC)]
            qT_r = [pp.tile([128, T], f32r, name=f"qT{h}", uniquify=True) for h in range(HPC)]
            v_sb = pp.tile([128, n_st, HPC * HD], f32r)  # [s-tile part, st, 2*128]

            # ---------------- phase 1: kv projection + rope(k) ----------------
            with tc.tile_pool(name="xmelp", bufs=NKT) as xp, \
                 tc.tile_pool(name="cs1", bufs=4) as csp, \
                 tc.tile_pool(name="rtmp1", bufs=4) as rtp, \
                 tc.tile_pool(name="ps_kv", bufs=2, space="PSUM") as pskv, \
                 tc.tile_pool(name="ps_v", bufs=2, space="PSUM") as psv:
                xm = []
                for kt in range(NKT):
                    xm_t = xp.tile([128, S], f32r, name=f"xm{kt}", uniquify=True, tag="xm", bufs=NKT)
                    nc.sync.dma_start(out=xm_t[:], in_=xmelT[kt * 128:(kt + 1) * 128, :].bitcast(f32r))
                    xm.append(xm_t)

                for h in range(HPC):
                    for (c0, cw) in s_chunks:
                        kps = pskv.tile([128, 512], f32, name="kps", tag="kps", bufs=2)
                        for kt in range(NKT):
                            nc.tensor.matmul(
                                kps[:, :cw],
                                wk[:, kt, h * HD:(h + 1) * HD],
                                xm[kt][:, c0:c0 + cw],
                                start=(kt == 0), stop=(kt == NKT - 1))
                        cos_sb = csp.tile([128, 512], f32, name="cos_sb", tag="cos", bufs=2)
                        sin_sb = csp.tile([128, 512], f32, name="sin_sb", tag="sin", bufs=2)
                        nc.sync.dma_start(out=cos_sb[:, :cw], in_=cosk[:, c0:c0 + cw])
                        nc.sync.dma_start(out=sin_sb[:, :cw], in_=sink[:, c0:c0 + cw])
                        out_sl = kT_r[h][:, c0:c0 + cw]
                        swp = rtp.tile([128, 512], f32, name="swp", tag="rt", bufs=3)
                        nc.vector.tensor_copy(swp[0:64, :cw], kps[64:128, :cw])
                        nc.vector.tensor_copy(swp[64:128, :cw], kps[0:64, :cw])
                        nc.vector.tensor_mul(swp[:, :cw], swp[:, :cw], sin_sb[:, :cw])
                        nc.vector.tensor_mul(out_sl, kps[:, :cw], cos_sb[:, :cw])
                        nc.vector.tensor_add(out_sl, out_sl, swp[:, :cw])

                for st in range(n_st):
                    s0 = st * 128
                    scnt = min(128, S - s0)
                    vps = psv.tile([128, HPC * HD], f32, name="vps", tag="vps", bufs=2)
                    for kt in range(NKT):
                        nc.tensor.matmul(
                            vps[:scnt, :],
                            xm[kt][:, s0:s0 + scnt],
                            wv[:, kt, :],
                            start=(kt == 0), stop=(kt == NKT - 1))
                    nc.vector.tensor_copy(v_sb[:scnt, st, :], vps[:scnt, :])

            # ---------------- phase 2: q projection + rope(q) ----------------
            with tc.tile_pool(name="xqp", bufs=NKT) as xqp, \
                 tc.tile_pool(name="cs2", bufs=4) as csp2, \
                 tc.tile_pool(name="rtmp2", bufs=4) as rtp2, \
                 tc.tile_pool(name="ps_q", bufs=2, space="PSUM") as psq:
                xq = []
                for kt in range(NKT):
                    xq_t = xqp.tile([128, T], f32r, name=f"xq{kt}", uniquify=True, tag="xq", bufs=NKT)
                    nc.sync.dma_start(out=xq_t[:], in_=xT[kt * 128:(kt + 1) * 128, :].bitcast(f32r))
                    xq.append(xq_t)

                for h in range(HPC):
                    for (c0, cw) in t_chunks:
                        qps = psq.tile([128, 512], f32, name="qps", tag="qps", bufs=2)
                        for kt in range(NKT):
                            nc.tensor.matmul(
                                qps[:, :cw],
                                wq[:, kt, h * HD:(h + 1) * HD],
                                xq[kt][:, c0:c0 + cw],
                                start=(kt == 0), stop=(kt == NKT - 1))
                        cos_sb = csp2.tile([128, 512], f32, name="cos_sb2", tag="cos2", bufs=2)
                        sin_sb = csp2.tile([128, 512], f32, name="sin_sb2", tag="sin2", bufs=2)
                        nc.sync.dma_start(out=cos_sb[:, :cw], in_=cosq[:, c0:c0 + cw])
                        nc.sync.dma_start(out=sin_sb[:, :cw], in_=sinq[:, c0:c0 + cw])
                        out_sl = qT_r[h][:, c0:c0 + cw]
                        swp = rtp2.tile([128, 512], f32, name="swp2", tag="qrt", bufs=3)
                        nc.vector.tensor_copy(swp[0:64, :cw], qps[64:128, :cw])
                        nc.vector.tensor_copy(swp[64:128, :cw], qps[0:64, :cw])
                        nc.vector.tensor_mul(swp[:, :cw], swp[:, :cw], sin_sb[:, :cw])
                        nc.vector.tensor_mul(out_sl, qps[:, :cw], cos_sb[:, :cw])
                        nc.vector.tensor_add(out_sl, out_sl, swp[:, :cw])

            # ---------------- phase 3: attention + out projection ----------------
            with tc.tile_pool(name="pP", bufs=4) as pP, \
                 tc.tile_pool(name="aoP", bufs=2 * HPC) as aoP, \
                 tc.tile_pool(name="zP", bufs=2) as zP, \
                 tc.tile_pool(name="yP", bufs=2) as yP, \
                 tc.tile_pool(name="ps_sc", bufs=2, space="PSUM") as ps_sc, \
                 tc.tile_pool(name="ps_o2", bufs=2, space="PSUM") as ps_o2, \
                 tc.tile_pool(name="ps_z", bufs=2, space="PSUM") as ps_z, \
                 tc.tile_pool(name="ps_y", bufs=1, space="PSUM") as ps_y:
                for (c0, cw) in t_chunks:
                    ao = []
                    for h in range(HPC):
                        o2ps = ps_o2.tile([128, 512], f32, name="o2ps", tag="o2", bufs=2)
                        zps = ps_z.tile([1, 512], f32, name="zps", tag="z", bufs=2)
                        for st in range(n_st):
                            s0 = st * 128
                            scnt = min(128, S - s0)
                            scps = ps_sc.tile([128, 512], f32, name="scps", tag="sc", bufs=2)
                            nc.tensor.matmul(
                                scps[:scnt, :cw],
                                kT_r[h][:, s0:s0 + scnt],
                                qT_r[h][:, c0:c0 + cw],
                                start=True, stop=True)
                            p_t = pP.tile([128, 512], f32r, name="p_t", tag="p", bufs=4)
                            nc.scalar.activation(p_t[:scnt, :cw], scps[:scnt, :cw],
                                                 mybir.ActivationFunctionType.Exp)
                            nc.tensor.matmul(
                                o2ps[:, :cw],
                                v_sb[:scnt, st, h * HD:(h + 1) * HD],
                                p_t[:scnt, :cw],
                                start=(st == 0), stop=(st == n_st - 1))
                            nc.tensor.matmul(
                                zps[:, :cw],
                                ones[:scnt, :],
                                p_t[:scnt, :cw],
                                start=(st == 0), stop=(st == n_st - 1))
                        recip = zP.tile([1, 512], f32, name="recip", tag="recip", bufs=2)
                        nc.vector.reciprocal(recip[:, :cw], zps[:, :cw])
                        zrow = h * len(t_chunks) + (c0 // 512)
                        nc.sync.dma_start(out=zsd[zrow:zrow + 1, :cw], in_=recip[:, :cw])
                        zrep = zP.tile([128, 512], f32, name="zrep", tag="zrep", bufs=2)
                        nc.sync.dma_start(out=zrep[:, :cw], in_=zsd[zrow, :cw].partition_broadcast(128))
                        ao_h = aoP.tile([128, 512], f32r, name=f"ao{h}", tag=f"ao{h}", bufs=2)
                        nc.vector.tensor_mul(ao_h[:, :cw], o2ps[:, :cw], zrep[:, :cw])
                        ao.append(ao_h)

                    for tt in range(cw // 128):
                        yps = ps_y.tile([128, DIM], f32, name="yps", tag="y", bufs=1)
                        for nn in range(DIM // 512):
                            for h in range(HPC):
                                nc.tensor.matmul(
                                    yps[:, nn * 512:(nn + 1) * 512],
                                    ao[h][:, tt * 128:(tt + 1) * 128],
                                    wo[h][:, nn * 512:(nn + 1) * 512],
                                    start=(h == 0), stop=(h == HPC - 1))
                        y_sb = yP.tile([128, DIM], f32, name="y_sb", tag="ysb", bufs=2)
                        nc.vector.tensor_copy(y_sb[:], yps[:])
                        nc.sync.dma_start(out=y[c0 + tt * 128: c0 + (tt + 1) * 128, :], in_=y_sb[:])

    nc.compile()
    return nc


def _host_tables(T=T, S=S):
    scale = float(HD) ** (-0.25)
    inv = 1.0 / (ROPE_BASE ** (np.arange(0, HD, 2, dtype=np.float64) / HD))  # [64]

    def tables(L):
        fr = np.outer(inv, np.arange(L, dtype=np.float64))  # [64, L]
        c = np.cos(fr) * scale
        s = np.sin(fr) * scale
        cos = np.concatenate([c, c], axis=0).astype(np.float32)
        sin = np.concatenate([-s, s], axis=0).astype(np.float32)
        return np.ascontiguousarray(cos), np.ascontiguousarray(sin)

    cosq_, sinq_ = tables(T)
    cosk_, sink_ = tables(S)
    return cosq_, sinq_, cosk_, sink_


def make_in_maps(x, xmel, Wq, Wkv, Wout):
    Bx, Tx, C = x.shape
    Sx = xmel.shape[1]
    cosq_, sinq_, cosk_, sink_ = _host_tables(Tx, Sx)

    x = np.asarray(x, dtype=np.float32)
    xmel = np.asarray(xmel, dtype=np.float32)
    Wq = np.asarray(Wq, dtype=np.float32)
    Wkv = np.asarray(Wkv, dtype=np.float32)
    Wout = np.asarray(Wout, dtype=np.float32)

    xT_b = [np.ascontiguousarray(x[b].T) for b in range(Bx)]
    xmelT_b = [np.ascontiguousarray(xmel[b].T) for b in range(Bx)]
    gsz = HPC * HD  # 256
    WqT_g, WkT_g, WvT_g, WoT_g = [], [], [], []
    for g in range(NG):
        r0 = g * gsz
        WqT_g.append(np.ascontiguousarray(Wq[r0:r0 + gsz, :].T))
        WkT_g.append(np.ascontiguousarray(Wkv[r0:r0 + gsz, :].T))
        WvT_g.append(np.ascontiguousarray(Wkv[DIM + r0:DIM + r0 + gsz, :].T))
        WoT_g.append(np.ascontiguousarray(Wout[:, r0:r0 + gsz].T))

    in_maps = []
    for c in range(Bx * NG):
        b, g = c // NG, c % NG
        in_maps.append({
            "xT": xT_b[b], "xmelT": xmelT_b[b],
            "WqT": WqT_g[g], "WkT": WkT_g[g], "WvT": WvT_g[g], "WoT": WoT_g[g],
            "cosq": cosq_, "sinq": sinq_, "cosk": cosk_, "sink": sink_,
        })
    return in_maps


def kernel(x, xmel, Wq, Wkv, Wout):
    from concourse.bass_utils import run_bass_kernel_spmd

    x = np.asarray(x, dtype=np.float32)
    xmel = np.asarray(xmel, dtype=np.float32)
    Bx, Tx, C = x.shape
    Sx = xmel.shape[1]
    assert (Bx, Tx, C, Sx) == (B, T, DIM, S)

    if "nc" not in _cache:
        _cache["nc"] = build_nc()
    nc = _cache["nc"]

    in_maps = make_in_maps(x, xmel,
                           np.asarray(Wq, dtype=np.float32),
                           np.asarray(Wkv, dtype=np.float32),
                           np.asarray(Wout, dtype=np.float32))
    res = run_bass_kernel_spmd(nc, in_maps, list(range(8)))
    out = np.zeros((B, T, DIM), dtype=np.float32)
    for c in range(8):
        b = c // NG
        out[b] += res.results[c]["y"]
    return out


# revision 14
# speedup vs baseline: 1.0023x; 1.0023x over previous
"""Trainium2 Bass kernel for MHA cross-attention (nn_MHACross).

Sharding: 8 cores = 2 batches x 4 head-groups (2 heads each).
Each core computes, for its (batch b, head group g):
    q = x[b] @ Wq[g].T ; k,v = xmel[b] @ Wkv[g].T ; RoPE(q, k) (scale folded
    into cos/sin tables); scores^T = k_r @ q_r^T per head; p = exp(scores)
    (no max subtraction -- scores are O(6) here, safe in fp32); unnormalized
    out2 = v^T @ p and Z = ones^T @ p via PE; normalize; y_partial = attn @
    Wout[:, g].T.  Host sums the 4 partial y per batch.

All matmuls use float32r (TF32-like: full speed at N>=256, ~1.5e-4 rel err).
Layouts keep the contraction dim on partitions everywhere, so there are no
on-device transposes:
    xT/xmelT      [C, T/S]   (pre-transposed on host)
    qT_r, kT_r    [D=128, T/S]  per head
    v             [S-tile=128, 2*D]  (both heads side by side, N=256 matmuls)
    scores^T      [S-tile, T-chunk=512]
    out2          [D, T-chunk] accumulated over S-tiles in PSUM
    Z             [1, T-chunk] accumulated via ones-matmul
    y             [T-tile=128, C]
"""
import sys
sys.path.insert(0, '/opt/trn_rl_repo')
import numpy as np

DIM = 1024
NHEADS = 8
HD = 128          # head dim
HPC = 2           # heads per core
NG = 4            # head groups (cores per batch)
B, T, S = 2, 2048, 3000
NKT = DIM // 128  # contraction tiles
ROPE_BASE = 10000.0
SWAP_MASK = [(i + 16) % 32 for i in range(32)]

_cache = {}


def _ceil_div(a, b):
    return (a + b - 1) // b


def build_nc(T=T, S=S):
    from concourse import bacc, mybir
    from concourse.tile import TileContext

    f32 = mybir.dt.float32
    f32r = mybir.dt.float32r
    bf16 = mybir.dt.bfloat16

    nc = bacc.Bacc("TRN2", target_bir_lowering=False, debug=False, num_devices=8)

    xT = nc.dram_tensor("xT", [DIM, T], f32, kind="ExternalInput")
    xmelT = nc.dram_tensor("xmelT", [DIM, S], f32, kind="ExternalInput")
    WqT = nc.dram_tensor("WqT", [DIM, HPC * HD], f32, kind="ExternalInput")
    WkT = nc.dram_tensor("WkT", [DIM, HPC * HD], f32, kind="ExternalInput")
    WvT = nc.dram_tensor("WvT", [DIM, HPC * HD], f32, kind="ExternalInput")
    WoT = nc.dram_tensor("WoT", [HPC * HD, DIM], f32, kind="ExternalInput")
    cosq = nc.dram_tensor("cosq", [HD, T], f32, kind="ExternalInput")
    sinq = nc.dram_tensor("sinq", [HD, T], f32, kind="ExternalInput")
    cosk = nc.dram_tensor("cosk", [HD, S], f32, kind="ExternalInput")
    sink = nc.dram_tensor("sink", [HD, S], f32, kind="ExternalInput")
    y = nc.dram_tensor("y", [T, DIM], f32, kind="ExternalOutput")
    # DRAM scratch for the Z-broadcast bounce (internal DRAM tensors fail to
    # load under the axon PJRT path, so declare it as an output)
    n_tc = _ceil_div(T, 512)
    zsd = nc.dram_tensor("zs", [HPC * n_tc, 512], f32, kind="ExternalOutput")

    n_st = _ceil_div(S, 128)                      # S-tiles (partition dim of scores^T)
    s_chunks = [(i * 512, min(512, S - i * 512)) for i in range(_ceil_div(S, 512))]
    t_chunks = [(i * 512, min(512, T - i * 512)) for i in range(_ceil_div(T, 512))]

    with TileContext(nc) as tc:
        with tc.tile_pool(name="wpool", bufs=1) as wp, \
             tc.tile_pool(name="persist", bufs=1) as pp:
            # persistent weights
            wq = wp.tile([128, NKT, HPC * HD], f32r)
            wk = wp.tile([128, NKT, HPC * HD], f32r)
            wv = wp.tile([128, NKT, HPC * HD], f32r)
            nc.sync.dma_start(out=wq[:], in_=WqT[:].rearrange("(k p) n -> p k n", p=128).bitcast(f32r))
            nc.sync.dma_start(out=wk[:], in_=WkT[:].rearrange("(k p) n -> p k n", p=128).bitcast(f32r))
            nc.sync.dma_start(out=wv[:], in_=WvT[:].rearrange("(k p) n -> p k n", p=128).bitcast(f32r))
            wo = []
            for h in range(HPC):
                wo_h = wp.tile([128, DIM], f32r, name=f"wo{h}", uniquify=True)
                nc.sync.dma_start(out=wo_h[:], in_=WoT[h * HD:(h + 1) * HD, :].bitcast(f32r))
                wo.append(wo_h)
            ones = wp.tile([128, 1], bf16)
            nc.vector.memset(ones[:], 1.0)

            # persistent activations
            kT_r = [pp.tile([128, S], f32r, name=f"kT{h}", uniquify=True) for h in range(HPC)]
            qT_r = [pp.tile([128, T], f32r, name=f"qT{h}", uniquify=True) for h in range(HPC)]
            v_sb = pp.tile([128, n_st, HPC * HD], bf16)  # [s-tile part, st, 2*128]

            # ---------------- phase 1: kv projection + rope(k) ----------------
            with tc.tile_pool(name="xmelp", bufs=NKT) as xp, \
                 tc.tile_pool(name="cs1", bufs=4) as csp, \
                 tc.tile_pool(name="rtmp1", bufs=4) as rtp, \
                 tc.tile_pool(name="ps_kv", bufs=2, space="PSUM") as pskv, \
                 tc.tile_pool(name="ps_v", bufs=2, space="PSUM") as psv:
                xm = []
                for kt in range(NKT):
                    xm_t = xp.tile([128, S], f32r, name=f"xm{kt}", uniquify=True, tag="xm", bufs=NKT)
                    nc.sync.dma_start(out=xm_t[:], in_=xmelT[kt * 128:(kt + 1) * 128, :].bitcast(f32r))
                    xm.append(xm_t)

                for h in range(HPC):
                    for (c0, cw) in s_chunks:
                        kps = pskv.tile([128, 512], f32, name="kps", tag="kps", bufs=2)
                        for kt in range(NKT):
                            nc.tensor.matmul(
                                kps[:, :cw],
                                wk[:, kt, h * HD:(h + 1) * HD],
                                xm[kt][:, c0:c0 + cw],
                                start=(kt == 0), stop=(kt == NKT - 1))
                        cos_sb = csp.tile([128, 512], f32, name="cos_sb", tag="cos", bufs=2)
                        sin_sb = csp.tile([128, 512], f32, name="sin_sb", tag="sin", bufs=2)
                        nc.sync.dma_start(out=cos_sb[:, :cw], in_=cosk[:, c0:c0 + cw])
                        nc.sync.dma_start(out=sin_sb[:, :cw], in_=sink[:, c0:c0 + cw])
                        out_sl = kT_r[h][:, c0:c0 + cw]
                        swp = rtp.tile([128, 512], f32, name="swp", tag="rt", bufs=3)
                        nc.vector.tensor_copy(swp[0:64, :cw], kps[64:128, :cw])
                        nc.vector.tensor_copy(swp[64:128, :cw], kps[0:64, :cw])
                        nc.vector.tensor_mul(swp[:, :cw], swp[:, :cw], sin_sb[:, :cw])
                        nc.vector.tensor_mul(out_sl, kps[:, :cw], cos_sb[:, :cw])
                        nc.vector.tensor_add(out_sl, out_sl, swp[:, :cw])

                for st in range(n_st):
                    s0 = st * 128
                    scnt = min(128, S - s0)
                    vps = psv.tile([128, HPC * HD], f32, name="vps", tag="vps", bufs=2)
                    for kt in range(NKT):
                        nc.tensor.matmul(
                            vps[:scnt, :],
                            xm[kt][:, s0:s0 + scnt],
                            wv[:, kt, :],
                            start=(kt == 0), stop=(kt == NKT - 1))
                    nc.vector.tensor_copy(v_sb[:scnt, st, :], vps[:scnt, :])

            # ---------------- phase 2: q projection + rope(q) ----------------
            with tc.tile_pool(name="xqp", bufs=NKT) as xqp, \
                 tc.tile_pool(name="cs2", bufs=4) as csp2, \
                 tc.tile_pool(name="rtmp2", bufs=4) as rtp2, \
                 tc.tile_pool(name="ps_q", bufs=2, space="PSUM") as psq:
                xq = []
                for kt in range(NKT):
                    xq_t = xqp.tile([128, T], f32r, name=f"xq{kt}", uniquify=True, tag="xq", bufs=NKT)
                    nc.sync.dma_start(out=xq_t[:], in_=xT[kt * 128:(kt + 1) * 128, :].bitcast(f32r))
                    xq.append(xq_t)

                for h in range(HPC):
                    for (c0, cw) in t_chunks:
                        qps = psq.tile([128, 512], f32, name="qps", tag="qps", bufs=2)
                        for kt in range(NKT):
                            nc.tensor.matmul(
                                qps[:, :cw],
                                wq[:, kt, h * HD:(h + 1) * HD],
                                xq[kt][:, c0:c0 + cw],
                                start=(kt == 0), stop=(kt == NKT - 1))
                        cos_sb = csp2.tile([128, 512], f32, name="cos_sb2", tag="cos2", bufs=2)
                        sin_sb = csp2.tile([128, 512], f32, name="sin_sb2", tag="sin2", bufs=2)
                        nc.sync.dma_start(out=cos_sb[:, :cw], in_=cosq[:, c0:c0 + cw])
                        nc.sync.dma_start(out=sin_sb[:, :cw], in_=sinq[:, c0:c0 + cw])
                        out_sl = qT_r[h][:, c0:c0 + cw]
                        swp = rtp2.tile([128, 512], f32, name="swp2", tag="qrt", bufs=3)
                        nc.vector.tensor_copy(swp[0:64, :cw], qps[64:128, :cw])
                        nc.vector.tensor_copy(swp[64:128, :cw], qps[0:64, :cw])
                        nc.vector.tensor_mul(swp[:, :cw], swp[:, :cw], sin_sb[:, :cw])
                        nc.vector.tensor_mul(out_sl, qps[:, :cw], cos_sb[:, :cw])
                        nc.vector.tensor_add(out_sl, out_sl, swp[:, :cw])

            # ---------------- phase 3: attention + out projection ----------------
            with tc.tile_pool(name="pP", bufs=4) as pP, \
                 tc.tile_pool(name="aoP", bufs=2 * HPC) as aoP, \
                 tc.tile_pool(name="zP", bufs=2) as zP, \
                 tc.tile_pool(name="yP", bufs=2) as yP, \
                 tc.tile_pool(name="ps_sc", bufs=2, space="PSUM") as ps_sc, \
                 tc.tile_pool(name="ps_o2", bufs=2, space="PSUM") as ps_o2, \
                 tc.tile_pool(name="ps_z", bufs=2, space="PSUM") as ps_z, \
                 tc.tile_pool(name="ps_y", bufs=1, space="PSUM") as ps_y:
                for (c0, cw) in t_chunks:
                    ao = []
                    for h in range(HPC):
                        o2ps = ps_o2.tile([128, 512], f32, name="o2ps", tag="o2", bufs=2)
                        zps = ps_z.tile([1, 512], f32, name="zps", tag="z", bufs=2)
                        # software pipeline: scores(st) issued one step ahead of
                        # attnV/z(st-1) so the PE keeps streaming while the
                        # scalar engine computes exp of the previous tile.
                        p_prev = None
                        prev_cnt = 0
                        for st in range(n_st + 1):
                            if st < n_st:
                                s0 = st * 128
                                scnt = min(128, S - s0)
                                scps = ps_sc.tile([128, 512], f32, name="scps", tag="sc", bufs=2)
                                nc.tensor.matmul(
                                    scps[:scnt, :cw],
                                    kT_r[h][:, s0:s0 + scnt],
                                    qT_r[h][:, c0:c0 + cw],
                                    start=True, stop=True)
                                p_t = pP.tile([128, 512], bf16, name="p_t", tag="p", bufs=4)
                                nc.scalar.activation(p_t[:scnt, :cw], scps[:scnt, :cw],
                                                     mybir.ActivationFunctionType.Exp)
                            if st > 0:
                                pst = st - 1
                                nc.tensor.matmul(
                                    o2ps[:, :cw],
                                    v_sb[:prev_cnt, pst, h * HD:(h + 1) * HD],
                                    p_prev[:prev_cnt, :cw],
                                    start=(pst == 0), stop=(pst == n_st - 1))
                                nc.tensor.matmul(
                                    zps[:, :cw],
                                    ones[:prev_cnt, :],
                                    p_prev[:prev_cnt, :cw],
                                    start=(pst == 0), stop=(pst == n_st - 1))
                            if st < n_st:
                                p_prev = p_t
                                prev_cnt = scnt
                        recip = zP.tile([1, 512], f32, name="recip", tag="recip", bufs=2)
                        nc.vector.reciprocal(recip[:, :cw], zps[:, :cw])
                        zrow = h * len(t_chunks) + (c0 // 512)
                        nc.sync.dma_start(out=zsd[zrow:zrow + 1, :cw], in_=recip[:, :cw])
                        zrep = zP.tile([128, 512], f32, name="zrep", tag="zrep", bufs=2)
                        nc.sync.dma_start(out=zrep[:, :cw], in_=zsd[zrow, :cw].partition_broadcast(128))
                        ao_h = aoP.tile([128, 512], f32r, name=f"ao{h}", tag=f"ao{h}", bufs=2)
                        nc.vector.tensor_mul(ao_h[:, :cw], o2ps[:, :cw], zrep[:, :cw])
                        ao.append(ao_h)

                    for tt in range(cw // 128):
                        y_sb = yP.tile([128, DIM], f32, name="y_sb", tag="ysb", bufs=2)
                        for nn in range(DIM // 512):
                            yps = ps_y.tile([128, 512], f32, name="yps", tag="y", bufs=2)
                            for h in range(HPC):
                                nc.tensor.matmul(
                                    yps[:],
                                    ao[h][:, tt * 128:(tt + 1) * 128],
                                    wo[h][:, nn * 512:(nn + 1) * 512],
                                    start=(h == 0), stop=(h == HPC - 1))
                            nc.vector.tensor_copy(y_sb[:, nn * 512:(nn + 1) * 512], yps[:])
                        nc.sync.dma_start(out=y[c0 + tt * 128: c0 + (tt + 1) * 128, :], in_=y_sb[:])

    nc.compile()
    return nc


def _host_tables(T=T, S=S):
    scale = float(HD) ** (-0.25)
    inv = 1.0 / (ROPE_BASE ** (np.arange(0, HD, 2, dtype=np.float64) / HD))  # [64]

    def tables(L):
        fr = np.outer(inv, np.arange(L, dtype=np.float64))  # [64, L]
        c = np.cos(fr) * scale
        s = np.sin(fr) * scale
        cos = np.concatenate([c, c], axis=0).astype(np.float32)
        sin = np.concatenate([-s, s], axis=0).astype(np.float32)
        return np.ascontiguousarray(cos), np.ascontiguousarray(sin)

    cosq_, sinq_ = tables(T)
    cosk_, sink_ = tables(S)
    return cosq_, sinq_, cosk_, sink_


def make_in_maps(x, xmel, Wq, Wkv, Wout):
    Bx, Tx, C = x.shape
    Sx = xmel.shape[1]
    cosq_, sinq_, cosk_, sink_ = _host_tables(Tx, Sx)

    x = np.asarray(x, dtype=np.float32)
    xmel = np.asarray(xmel, dtype=np.float32)
    Wq = np.asarray(Wq, dtype=np.float32)
    Wkv = np.asarray(Wkv, dtype=np.float32)
    Wout = np.asarray(Wout, dtype=np.float32)

    xT_b = [np.ascontiguousarray(x[b].T) for b in range(Bx)]
    xmelT_b = [np.ascontiguousarray(xmel[b].T) for b in range(Bx)]
    gsz = HPC * HD  # 256
    WqT_g, WkT_g, WvT_g, WoT_g = [], [], [], []
    for g in range(NG):
        r0 = g * gsz
        WqT_g.append(np.ascontiguousarray(Wq[r0:r0 + gsz, :].T))
        WkT_g.append(np.ascontiguousarray(Wkv[r0:r0 + gsz, :].T))
        WvT_g.append(np.ascontiguousarray(Wkv[DIM + r0:DIM + r0 + gsz, :].T))
        WoT_g.append(np.ascontiguousarray(Wout[:, r0:r0 + gsz].T))

    in_maps = []
    for c in range(Bx * NG):
        b, g = c // NG, c % NG
        in_maps.append({
            "xT": xT_b[b], "xmelT": xmelT_b[b],
            "WqT": WqT_g[g], "WkT": WkT_g[g], "WvT": WvT_g[g], "WoT": WoT_g[g],
            "cosq": cosq_, "sinq": sinq_, "cosk": cosk_, "sink": sink_,
        })
    return in_maps


def kernel(x, xmel, Wq, Wkv, Wout):
    from concourse.bass_utils import run_bass_kernel_spmd

    x = np.asarray(x, dtype=np.float32)
    xmel = np.asarray(xmel, dtype=np.float32)
    Bx, Tx, C = x.shape
    Sx = xmel.shape[1]
    assert (Bx, Tx, C, Sx) == (B, T, DIM, S)

    if "nc" not in _cache:
        _cache["nc"] = build_nc()
    nc = _cache["nc"]

    in_maps = make_in_maps(x, xmel,
                           np.asarray(Wq, dtype=np.float32),
                           np.asarray(Wkv, dtype=np.float32),
                           np.asarray(Wout, dtype=np.float32))
    res = run_bass_kernel_spmd(nc, in_maps, list(range(8)))
    out = np.zeros((B, T, DIM), dtype=np.float32)
    for c in range(8):
        b = c // NG
        out[b] += res.results[c]["y"]
    return out


# revision 16
# speedup vs baseline: 1.1561x; 1.1534x over previous
"""Trainium2 Bass kernel for MHA cross-attention (nn_MHACross).

Sharding: 8 cores = 2 batches x 4 head-groups (2 heads each).
Each core computes, for its (batch b, head group g):
    q = x[b] @ Wq[g].T ; k,v = xmel[b] @ Wkv[g].T ; RoPE(q, k) (scale folded
    into host-side cos/sin tables); per head scores^T = k_r @ q_r^T;
    p = exp(scores) with no max subtraction (scores are O(6) here, safe in
    fp32); unnormalized out2 = v^T @ p and Z = ones^T @ p on the PE;
    normalize by 1/Z; y_partial = attn @ Wout[:, g].T.  Host sums the 4
    partial y per batch.

Layouts keep the contraction dim on partitions everywhere; no on-device
transposes.  Matmul operands are bf16 (except the final projection, which
runs in float32r); PSUM accumulation is fp32 throughout.  The attention
inner loop is batched by op type (all scores, then all attnV, then all Z
matmuls per head/chunk-pair) so the PE streams back-to-back with stationary
reuse, while exp for both 512-chunks of a pair runs as one [128,1024]
scalar-engine instruction.
"""
import sys
sys.path.insert(0, '/opt/trn_rl_repo')
import numpy as np

DIM = 1024
NHEADS = 8
HD = 128          # head dim
HPC = 2           # heads per core
NG = 4            # head groups (cores per batch)
B, T, S = 2, 2048, 3000
NKT = DIM // 128  # contraction tiles
ROPE_BASE = 10000.0
CW = 512          # T-chunk width
PAIR = 2 * CW     # paired chunk width for exp

_cache = {}


def _ceil_div(a, b):
    return (a + b - 1) // b


def build_nc(T=T, S=S):
    from concourse import bacc, mybir
    from concourse.tile import TileContext

    f32 = mybir.dt.float32
    f32r = mybir.dt.float32r
    bf16 = mybir.dt.bfloat16

    nc = bacc.Bacc("TRN2", target_bir_lowering=False, debug=False, num_devices=8)

    xT = nc.dram_tensor("xT", [DIM, T], bf16, kind="ExternalInput")
    xmelT = nc.dram_tensor("xmelT", [DIM, S], bf16, kind="ExternalInput")
    WqT = nc.dram_tensor("WqT", [DIM, HPC * HD], bf16, kind="ExternalInput")
    WkT = nc.dram_tensor("WkT", [DIM, HPC * HD], bf16, kind="ExternalInput")
    WvT = nc.dram_tensor("WvT", [DIM, HPC * HD], bf16, kind="ExternalInput")
    WoT = nc.dram_tensor("WoT", [HPC * HD, DIM], f32, kind="ExternalInput")
    cosq = nc.dram_tensor("cosq", [HD, T], f32, kind="ExternalInput")
    sinq = nc.dram_tensor("sinq", [HD, T], f32, kind="ExternalInput")
    cosk = nc.dram_tensor("cosk", [HD, S], f32, kind="ExternalInput")
    sink = nc.dram_tensor("sink", [HD, S], f32, kind="ExternalInput")
    y = nc.dram_tensor("y", [T, DIM], f32, kind="ExternalOutput")
    # DRAM scratch for the Z-broadcast bounce (internal DRAM tensors fail to
    # load under the axon PJRT path, so declare it as an output)
    n_tc = _ceil_div(T, CW)
    zsd = nc.dram_tensor("zs", [HPC * n_tc, CW], f32, kind="ExternalOutput")

    n_st = _ceil_div(S, 128)
    s_chunks = [(i * 512, min(512, S - i * 512)) for i in range(_ceil_div(S, 512))]
    t_chunks = [(i * CW, min(CW, T - i * CW)) for i in range(n_tc)]
    t_pairs = [t_chunks[i:i + 2] for i in range(0, n_tc, 2)]

    with TileContext(nc) as tc:
        with tc.tile_pool(name="wpool", bufs=1) as wp, \
             tc.tile_pool(name="persist", bufs=1) as pp:
            # persistent weights
            wq = wp.tile([128, NKT, HPC * HD], bf16)
            wk = wp.tile([128, NKT, HPC * HD], bf16)
            wv = wp.tile([128, NKT, HPC * HD], bf16)
            wo = []
            for h in range(HPC):
                wo_h = wp.tile([128, DIM], f32r, name=f"wo{h}", uniquify=True)
                nc.sync.dma_start(out=wo_h[:], in_=WoT[h * HD:(h + 1) * HD, :].bitcast(f32r))
                wo.append(wo_h)
            ones = wp.tile([128, 1], bf16)
            nc.vector.memset(ones[:], 1.0)

            # persistent activations
            kT_r = [pp.tile([128, S], bf16, name=f"kT{h}", uniquify=True) for h in range(HPC)]
            qT_r = [pp.tile([128, T], bf16, name=f"qT{h}", uniquify=True) for h in range(HPC)]
            v_sb = pp.tile([128, n_st, HPC * HD], bf16)

            # ------------- phase 1: projections + rope (q first) -------------
            with tc.tile_pool(name="xqp", bufs=NKT) as xqp, \
                 tc.tile_pool(name="xmelp", bufs=NKT) as xp, \
                 tc.tile_pool(name="csP", bufs=4) as csp, \
                 tc.tile_pool(name="rtP", bufs=3) as rtp, \
                 tc.tile_pool(name="ps_pr", bufs=3, space="PSUM") as pspr, \
                 tc.tile_pool(name="ps_v", bufs=2, space="PSUM") as psv:
                # x first (smaller), so q-proj compute overlaps the xmel DMA
                nc.sync.dma_start(out=wq[:], in_=WqT[:].rearrange("(k p) n -> p k n", p=128))
                nc.sync.dma_start(out=wk[:], in_=WkT[:].rearrange("(k p) n -> p k n", p=128))
                nc.sync.dma_start(out=wv[:], in_=WvT[:].rearrange("(k p) n -> p k n", p=128))
                xq = []
                for kt in range(NKT):
                    xq_t = xqp.tile([128, T], bf16, name=f"xq{kt}", uniquify=True, tag="xq", bufs=NKT)
                    nc.sync.dma_start(out=xq_t[:], in_=xT[kt * 128:(kt + 1) * 128, :])
                    xq.append(xq_t)
                xm = []
                for kt in range(NKT):
                    xm_t = xp.tile([128, S], bf16, name=f"xm{kt}", uniquify=True, tag="xm", bufs=NKT)
                    nc.sync.dma_start(out=xm_t[:], in_=xmelT[kt * 128:(kt + 1) * 128, :])
                    xm.append(xm_t)

                def proj_rope(h, c0, cw, w_sb, src, cos_d, sin_d, out_sl):
                    ps = pspr.tile([128, 512], f32, name="prps", tag="prps", bufs=3)
                    for kt in range(NKT):
                        nc.tensor.matmul(
                            ps[:, :cw],
                            w_sb[:, kt, h * HD:(h + 1) * HD],
                            src[kt][:, c0:c0 + cw],
                            start=(kt == 0), stop=(kt == NKT - 1))
                    cos_sb = csp.tile([128, 512], f32, name="cos_sb", tag="cos", bufs=2)
                    sin_sb = csp.tile([128, 512], f32, name="sin_sb", tag="sin", bufs=2)
                    nc.sync.dma_start(out=cos_sb[:, :cw], in_=cos_d[:, c0:c0 + cw])
                    nc.sync.dma_start(out=sin_sb[:, :cw], in_=sin_d[:, c0:c0 + cw])
                    swp = rtp.tile([128, 512], f32, name="swp", tag="rt", bufs=3)
                    nc.vector.tensor_copy(swp[0:64, :cw], ps[64:128, :cw])
                    nc.vector.tensor_copy(swp[64:128, :cw], ps[0:64, :cw])
                    nc.vector.tensor_mul(swp[:, :cw], swp[:, :cw], sin_sb[:, :cw])
                    nc.vector.tensor_mul(out_sl, ps[:, :cw], cos_sb[:, :cw])
                    nc.vector.tensor_add(out_sl, out_sl, swp[:, :cw])

                for h in range(HPC):
                    for (c0, cw) in t_chunks:
                        proj_rope(h, c0, cw, wq, xq, cosq, sinq, qT_r[h][:, c0:c0 + cw])
                for h in range(HPC):
                    for (c0, cw) in s_chunks:
                        proj_rope(h, c0, cw, wk, xm, cosk, sink, kT_r[h][:, c0:c0 + cw])
                for st in range(n_st):
                    s0 = st * 128
                    scnt = min(128, S - s0)
                    vps = psv.tile([128, HPC * HD], f32, name="vps", tag="vps", bufs=2)
                    for kt in range(NKT):
                        nc.tensor.matmul(
                            vps[:scnt, :],
                            xm[kt][:, s0:s0 + scnt],
                            wv[:, kt, :],
                            start=(kt == 0), stop=(kt == NKT - 1))
                    nc.vector.tensor_copy(v_sb[:scnt, st, :], vps[:scnt, :])

            # ------------- phase 2: attention + out projection -------------
            with tc.tile_pool(name="pP", bufs=n_st + 2) as pP, \
                 tc.tile_pool(name="aoP", bufs=2 * HPC + 2) as aoP, \
                 tc.tile_pool(name="zP", bufs=4) as zP, \
                 tc.tile_pool(name="yP", bufs=2) as yP, \
                 tc.tile_pool(name="ps_sc", bufs=2, space="PSUM") as ps_sc, \
                 tc.tile_pool(name="ps_o2", bufs=2, space="PSUM") as ps_o2, \
                 tc.tile_pool(name="ps_z", bufs=2, space="PSUM") as ps_z:
                for pair in t_pairs:
                    pw = sum(cw for _, cw in pair)       # total pair width
                    p0 = pair[0][0]                      # pair base column
                    ao = {}
                    for h in range(HPC):
                        # ---- scores + exp, batched over S-tiles ----
                        ptiles = []
                        for st in range(n_st):
                            s0 = st * 128
                            scnt = min(128, S - s0)
                            scps = ps_sc.tile([128, PAIR], f32, name="scps", tag="sc", bufs=2)
                            for ci, (c0, cw) in enumerate(pair):
                                nc.tensor.matmul(
                                    scps[:scnt, ci * CW: ci * CW + cw],
                                    kT_r[h][:, s0:s0 + scnt],
                                    qT_r[h][:, c0:c0 + cw],
                                    start=True, stop=True,
                                    skip_group_check=True)
                            p_t = pP.tile([128, PAIR], bf16, name="p_t", tag="p", bufs=n_st + 2)
                            nc.scalar.activation(p_t[:scnt, :pw], scps[:scnt, :pw],
                                                 mybir.ActivationFunctionType.Exp)
                            ptiles.append((p_t, scnt))
                        # ---- attnV, batched (v stationary amortized) ----
                        o2 = []
                        for ci, (c0, cw) in enumerate(pair):
                            o2ps = ps_o2.tile([128, CW], f32, name="o2ps", tag="o2", bufs=2)
                            o2.append(o2ps)
                        for st in range(n_st):
                            p_t, scnt = ptiles[st]
                            for ci, (c0, cw) in enumerate(pair):
                                nc.tensor.matmul(
                                    o2[ci][:, :cw],
                                    v_sb[:scnt, st, h * HD:(h + 1) * HD],
                                    p_t[:scnt, ci * CW: ci * CW + cw],
                                    start=(st == 0), stop=(st == n_st - 1))
                        # ---- Z, batched (ones stationary loaded once) ----
                        zps = []
                        for ci, (c0, cw) in enumerate(pair):
                            z_ps = ps_z.tile([1, CW], f32, name="zps", tag="z", bufs=2)
                            zps.append(z_ps)
                        for st in range(n_st):
                            p_t, scnt = ptiles[st]
                            for ci, (c0, cw) in enumerate(pair):
                                nc.tensor.matmul(
                                    zps[ci][:, :cw],
                                    ones[:scnt, :],
                                    p_t[:scnt, ci * CW: ci * CW + cw],
                                    start=(st == 0), stop=(st == n_st - 1))
                        # ---- normalize ----
                        for ci, (c0, cw) in enumerate(pair):
                            recip = zP.tile([1, CW], f32, name="recip", tag="recip", bufs=2)
                            nc.vector.reciprocal(recip[:, :cw], zps[ci][:, :cw])
                            zrow = h * n_tc + (c0 // CW)
                            nc.sync.dma_start(out=zsd[zrow:zrow + 1, :cw], in_=recip[:, :cw])
                            zrep = zP.tile([128, CW], f32, name="zrep", tag="zrep", bufs=2)
                            nc.sync.dma_start(out=zrep[:, :cw], in_=zsd[zrow, :cw].partition_broadcast(128))
                            ao_h = aoP.tile([128, CW], f32r, name="ao", tag="ao", bufs=2 * HPC + 2)
                            nc.vector.tensor_mul(ao_h[:, :cw], o2[ci][:, :cw], zrep[:, :cw])
                            ao[(h, ci)] = ao_h
                    # ---- out projection for this pair (PSUM slots reused from sc pool) ----
                    for ci, (c0, cw) in enumerate(pair):
                        for tt in range(cw // 128):
                            y_sb = yP.tile([128, DIM], f32, name="y_sb", tag="ysb", bufs=2)
                            for nn in range(DIM // 512):
                                yps = ps_sc.tile([128, 512], f32, name="yps", tag="sc", bufs=2)
                                for h in range(HPC):
                                    nc.tensor.matmul(
                                        yps[:],
                                        ao[(h, ci)][:, tt * 128:(tt + 1) * 128],
                                        wo[h][:, nn * 512:(nn + 1) * 512],
                                        start=(h == 0), stop=(h == HPC - 1))
                                nc.vector.tensor_copy(y_sb[:, nn * 512:(nn + 1) * 512], yps[:])
                            nc.sync.dma_start(out=y[c0 + tt * 128: c0 + (tt + 1) * 128, :], in_=y_sb[:])

    nc.compile()
    return nc


def _host_tables(T=T, S=S):
    scale = float(HD) ** (-0.25)
    inv = 1.0 / (ROPE_BASE ** (np.arange(0, HD, 2, dtype=np.float64) / HD))  # [64]

    def tables(L):
        fr = np.outer(inv, np.arange(L, dtype=np.float64))  # [64, L]
        c = np.cos(fr) * scale
        s = np.sin(fr) * scale
        cos = np.concatenate([c, c], axis=0).astype(np.float32)
        sin = np.concatenate([-s, s], axis=0).astype(np.float32)
        return np.ascontiguousarray(cos), np.ascontiguousarray(sin)

    cosq_, sinq_ = tables(T)
    cosk_, sink_ = tables(S)
    return cosq_, sinq_, cosk_, sink_


def make_in_maps(x, xmel, Wq, Wkv, Wout):
    import ml_dtypes
    bf = ml_dtypes.bfloat16
    Bx, Tx, C = x.shape
    Sx = xmel.shape[1]
    cosq_, sinq_, cosk_, sink_ = _host_tables(Tx, Sx)

    x = np.asarray(x, dtype=np.float32)
    xmel = np.asarray(xmel, dtype=np.float32)
    Wq = np.asarray(Wq, dtype=np.float32)
    Wkv = np.asarray(Wkv, dtype=np.float32)
    Wout = np.asarray(Wout, dtype=np.float32)

    xT_b = [np.ascontiguousarray(x[b].T).astype(bf) for b in range(Bx)]
    xmelT_b = [np.ascontiguousarray(xmel[b].T).astype(bf) for b in range(Bx)]
    gsz = HPC * HD  # 256
    WqT_g, WkT_g, WvT_g, WoT_g = [], [], [], []
    for g in range(NG):
        r0 = g * gsz
        WqT_g.append(np.ascontiguousarray(Wq[r0:r0 + gsz, :].T).astype(bf))
        WkT_g.append(np.ascontiguousarray(Wkv[r0:r0 + gsz, :].T).astype(bf))
        WvT_g.append(np.ascontiguousarray(Wkv[DIM + r0:DIM + r0 + gsz, :].T).astype(bf))
        WoT_g.append(np.ascontiguousarray(Wout[:, r0:r0 + gsz].T))

    in_maps = []
    for c in range(Bx * NG):
        b, g = c // NG, c % NG
        in_maps.append({
            "xT": xT_b[b], "xmelT": xmelT_b[b],
            "WqT": WqT_g[g], "WkT": WkT_g[g], "WvT": WvT_g[g], "WoT": WoT_g[g],
            "cosq": cosq_, "sinq": sinq_, "cosk": cosk_, "sink": sink_,
        })
    return in_maps


def kernel(x, xmel, Wq, Wkv, Wout):
    from concourse.bass_utils import run_bass_kernel_spmd

    x = np.asarray(x, dtype=np.float32)
    xmel = np.asarray(xmel, dtype=np.float32)
    Bx, Tx, C = x.shape
    Sx = xmel.shape[1]
    assert (Bx, Tx, C, Sx) == (B, T, DIM, S)

    if "nc" not in _cache:
        _cache["nc"] = build_nc()
    nc = _cache["nc"]

    in_maps = make_in_maps(x, xmel,
                           np.asarray(Wq, dtype=np.float32),
                           np.asarray(Wkv, dtype=np.float32),
                           np.asarray(Wout, dtype=np.float32))
    res = run_bass_kernel_spmd(nc, in_maps, list(range(8)))
    out = np.zeros((B, T, DIM), dtype=np.float32)
    for c in range(8):
        b = c // NG
        out[b] += res.results[c]["y"]
    return out


# revision 18
# speedup vs baseline: 1.2124x; 1.0486x over previous
"""Trainium2 Bass kernel for MHA cross-attention (nn_MHACross).

Sharding: 8 cores = 2 batches x 4 head-groups (2 heads each).
Each core computes, for its (batch b, head group g):
    q = x[b] @ Wq[g].T ; k,v = xmel[b] @ Wkv[g].T ; RoPE(q, k) (scale folded
    into host-side cos/sin tables); per head scores^T = k_r @ q_r^T;
    p = exp(scores) with no max subtraction (scores are O(6) here, safe in
    fp32); unnormalized out2 = v^T @ p and Z = ones^T @ p on the PE;
    normalize by 1/Z; y_partial = attn @ Wout[:, g].T.  Host sums the 4
    partial y per batch.

Layouts keep the contraction dim on partitions everywhere; no on-device
transposes.  Matmul operands are bf16 (except the final projection, which
runs in float32r); PSUM accumulation is fp32 throughout.  The attention
inner loop is batched by op type (all scores, then all attnV, then all Z
matmuls per head/chunk-pair) so the PE streams back-to-back with stationary
reuse, while exp for both 512-chunks of a pair runs as one [128,1024]
scalar-engine instruction.
"""
import sys
sys.path.insert(0, '/opt/trn_rl_repo')
import numpy as np

DIM = 1024
NHEADS = 8
HD = 128          # head dim
HPC = 2           # heads per core
NG = 4            # head groups (cores per batch)
B, T, S = 2, 2048, 3000
NKT = DIM // 128  # contraction tiles
ROPE_BASE = 10000.0
CW = 512          # T-chunk width
PAIR = 2 * CW     # paired chunk width for exp

_cache = {}


def _ceil_div(a, b):
    return (a + b - 1) // b


def build_nc(T=T, S=S):
    from concourse import bacc, mybir
    from concourse.tile import TileContext

    f32 = mybir.dt.float32
    f32r = mybir.dt.float32r
    bf16 = mybir.dt.bfloat16

    nc = bacc.Bacc("TRN2", target_bir_lowering=False, debug=False, num_devices=8)

    xT = nc.dram_tensor("xT", [DIM, T], bf16, kind="ExternalInput")
    xmelT = nc.dram_tensor("xmelT", [DIM, S], bf16, kind="ExternalInput")
    WqT = nc.dram_tensor("WqT", [DIM, HPC * HD], bf16, kind="ExternalInput")
    WkT = nc.dram_tensor("WkT", [DIM, HPC * HD], bf16, kind="ExternalInput")
    WvT = nc.dram_tensor("WvT", [DIM, HPC * HD], bf16, kind="ExternalInput")
    WoT = nc.dram_tensor("WoT", [HPC * HD, DIM], f32, kind="ExternalInput")
    cosq = nc.dram_tensor("cosq", [HD, T], f32, kind="ExternalInput")
    sinq = nc.dram_tensor("sinq", [HD, T], f32, kind="ExternalInput")
    cosk = nc.dram_tensor("cosk", [HD, S], f32, kind="ExternalInput")
    sink = nc.dram_tensor("sink", [HD, S], f32, kind="ExternalInput")
    y = nc.dram_tensor("y", [T, DIM], f32, kind="ExternalOutput")
    # DRAM scratch for the Z-broadcast bounce (internal DRAM tensors fail to
    # load under the axon PJRT path, so declare it as an output)
    n_tc = _ceil_div(T, CW)
    zsd = nc.dram_tensor("zs", [HPC * n_tc, CW], f32, kind="ExternalOutput")

    n_st = _ceil_div(S, 128)
    s_chunks = [(i * 512, min(512, S - i * 512)) for i in range(_ceil_div(S, 512))]
    t_chunks = [(i * CW, min(CW, T - i * CW)) for i in range(n_tc)]
    t_pairs = [t_chunks[i:i + 2] for i in range(0, n_tc, 2)]

    with TileContext(nc) as tc:
        with tc.tile_pool(name="wpool", bufs=1) as wp, \
             tc.tile_pool(name="persist", bufs=1) as pp:
            # persistent weights
            wq = wp.tile([128, NKT, HPC * HD], bf16)
            wk = wp.tile([128, NKT, HPC * HD], bf16)
            wv = wp.tile([128, NKT, HPC * HD], bf16)
            wo = []
            for h in range(HPC):
                wo_h = wp.tile([128, DIM], f32r, name=f"wo{h}", uniquify=True)
                nc.sync.dma_start(out=wo_h[:], in_=WoT[h * HD:(h + 1) * HD, :].bitcast(f32r))
                wo.append(wo_h)
            ones = wp.tile([128, 1], bf16)
            nc.vector.memset(ones[:], 1.0)

            # persistent activations
            kT_r = [pp.tile([128, S], bf16, name=f"kT{h}", uniquify=True) for h in range(HPC)]
            qT_r = [pp.tile([128, T], bf16, name=f"qT{h}", uniquify=True) for h in range(HPC)]
            v_sb = pp.tile([128, n_st, HPC * HD], bf16)

            # ------------- phase 1: projections + rope (q first) -------------
            with tc.tile_pool(name="xqp", bufs=NKT) as xqp, \
                 tc.tile_pool(name="xmelp", bufs=NKT) as xp, \
                 tc.tile_pool(name="csP", bufs=4) as csp, \
                 tc.tile_pool(name="rtP", bufs=3) as rtp, \
                 tc.tile_pool(name="ps_pr", bufs=3, space="PSUM") as pspr, \
                 tc.tile_pool(name="ps_v", bufs=2, space="PSUM") as psv:
                # x first (smaller), so q-proj compute overlaps the xmel DMA
                nc.sync.dma_start(out=wq[:], in_=WqT[:].rearrange("(k p) n -> p k n", p=128))
                nc.sync.dma_start(out=wk[:], in_=WkT[:].rearrange("(k p) n -> p k n", p=128))
                nc.sync.dma_start(out=wv[:], in_=WvT[:].rearrange("(k p) n -> p k n", p=128))
                xq = []
                for kt in range(NKT):
                    xq_t = xqp.tile([128, T], bf16, name=f"xq{kt}", uniquify=True, tag="xq", bufs=NKT)
                    nc.sync.dma_start(out=xq_t[:], in_=xT[kt * 128:(kt + 1) * 128, :])
                    xq.append(xq_t)
                xm = []
                for kt in range(NKT):
                    xm_t = xp.tile([128, S], bf16, name=f"xm{kt}", uniquify=True, tag="xm", bufs=NKT)
                    nc.sync.dma_start(out=xm_t[:], in_=xmelT[kt * 128:(kt + 1) * 128, :])
                    xm.append(xm_t)

                def proj_rope(h, c0, cw, w_sb, src, cos_d, sin_d, out_sl):
                    ps = pspr.tile([128, 512], f32, name="prps", tag="prps", bufs=3)
                    for kt in range(NKT):
                        nc.tensor.matmul(
                            ps[:, :cw],
                            w_sb[:, kt, h * HD:(h + 1) * HD],
                            src[kt][:, c0:c0 + cw],
                            start=(kt == 0), stop=(kt == NKT - 1))
                    cos_sb = csp.tile([128, 512], f32, name="cos_sb", tag="cos", bufs=2)
                    sin_sb = csp.tile([128, 512], f32, name="sin_sb", tag="sin", bufs=2)
                    nc.sync.dma_start(out=cos_sb[:, :cw], in_=cos_d[:, c0:c0 + cw])
                    nc.sync.dma_start(out=sin_sb[:, :cw], in_=sin_d[:, c0:c0 + cw])
                    swp = rtp.tile([128, 512], f32, name="swp", tag="rt", bufs=3)
                    nc.scalar.copy(swp[0:64, :cw], ps[64:128, :cw])
                    nc.scalar.copy(swp[64:128, :cw], ps[0:64, :cw])
                    nc.vector.tensor_mul(swp[:, :cw], swp[:, :cw], sin_sb[:, :cw])
                    nc.vector.tensor_mul(out_sl, ps[:, :cw], cos_sb[:, :cw])
                    nc.vector.tensor_add(out_sl, out_sl, swp[:, :cw])

                for h in range(HPC):
                    for (c0, cw) in t_chunks:
                        proj_rope(h, c0, cw, wq, xq, cosq, sinq, qT_r[h][:, c0:c0 + cw])
                for h in range(HPC):
                    for (c0, cw) in s_chunks:
                        proj_rope(h, c0, cw, wk, xm, cosk, sink, kT_r[h][:, c0:c0 + cw])
                for st in range(n_st):
                    s0 = st * 128
                    scnt = min(128, S - s0)
                    vps = psv.tile([128, HPC * HD], f32, name="vps", tag="vps", bufs=2)
                    for kt in range(NKT):
                        nc.tensor.matmul(
                            vps[:scnt, :],
                            xm[kt][:, s0:s0 + scnt],
                            wv[:, kt, :],
                            start=(kt == 0), stop=(kt == NKT - 1))
                    nc.vector.tensor_copy(v_sb[:scnt, st, :], vps[:scnt, :])

            # ------------- phase 2: attention + out projection -------------
            with tc.tile_pool(name="pP", bufs=n_st + 2) as pP, \
                 tc.tile_pool(name="aoP", bufs=2 * HPC + 2) as aoP, \
                 tc.tile_pool(name="zP", bufs=4) as zP, \
                 tc.tile_pool(name="yP", bufs=2) as yP, \
                 tc.tile_pool(name="ps_sc", bufs=2, space="PSUM") as ps_sc, \
                 tc.tile_pool(name="ps_o2", bufs=2, space="PSUM") as ps_o2, \
                 tc.tile_pool(name="ps_z", bufs=2, space="PSUM") as ps_z:
                for pair in t_pairs:
                    pw = sum(cw for _, cw in pair)       # total pair width
                    p0 = pair[0][0]                      # pair base column
                    ao = {}
                    for h in range(HPC):
                        # ---- scores + exp, batched over S-tiles ----
                        ptiles = []
                        for st in range(n_st):
                            s0 = st * 128
                            scnt = min(128, S - s0)
                            scps = ps_sc.tile([128, PAIR], f32, name="scps", tag="sc", bufs=2)
                            for ci, (c0, cw) in enumerate(pair):
                                nc.tensor.matmul(
                                    scps[:scnt, ci * CW: ci * CW + cw],
                                    kT_r[h][:, s0:s0 + scnt],
                                    qT_r[h][:, c0:c0 + cw],
                                    start=True, stop=True,
                                    skip_group_check=True)
                            p_t = pP.tile([128, PAIR], bf16, name="p_t", tag="p", bufs=n_st + 2)
                            nc.scalar.activation(p_t[:scnt, :pw], scps[:scnt, :pw],
                                                 mybir.ActivationFunctionType.Exp)
                            ptiles.append((p_t, scnt))
                        # ---- Z first, batched (ones stationary loaded once) ----
                        zps = []
                        for ci, (c0, cw) in enumerate(pair):
                            z_ps = ps_z.tile([1, CW], f32, name="zps", tag="z", bufs=2)
                            zps.append(z_ps)
                        for st in range(n_st):
                            p_t, scnt = ptiles[st]
                            for ci, (c0, cw) in enumerate(pair):
                                nc.tensor.matmul(
                                    zps[ci][:, :cw],
                                    ones[:scnt, :],
                                    p_t[:scnt, ci * CW: ci * CW + cw],
                                    start=(st == 0), stop=(st == n_st - 1))
                        # 1/Z computed while the attnV matmuls stream below
                        zr2s = []
                        for ci, (c0, cw) in enumerate(pair):
                            zrow = h * n_tc + (c0 // CW)
                            zsb = zP.tile([1, CW], f32, name="zsb", tag="zsb", bufs=2)
                            nc.vector.tensor_copy(zsb[:, :cw], zps[ci][:, :cw])
                            nc.sync.dma_start(out=zsd[zrow:zrow + 1, :cw], in_=zsb[:, :cw])
                            zrep = zP.tile([128, CW], f32, name="zrep", tag="zrep", bufs=2)
                            nc.sync.dma_start(out=zrep[:, :cw], in_=zsd[zrow, :cw].partition_broadcast(128))
                            zr2 = zP.tile([128, CW], f32, name="zr2", tag="zr2", bufs=2)
                            nc.vector.reciprocal(zr2[:, :cw], zrep[:, :cw])
                            zr2s.append(zr2)
                        # ---- attnV, batched (v stationary amortized) ----
                        o2 = []
                        for ci, (c0, cw) in enumerate(pair):
                            o2ps = ps_o2.tile([128, CW], f32, name="o2ps", tag="o2", bufs=2)
                            o2.append(o2ps)
                        for st in range(n_st):
                            p_t, scnt = ptiles[st]
                            for ci, (c0, cw) in enumerate(pair):
                                nc.tensor.matmul(
                                    o2[ci][:, :cw],
                                    v_sb[:scnt, st, h * HD:(h + 1) * HD],
                                    p_t[:scnt, ci * CW: ci * CW + cw],
                                    start=(st == 0), stop=(st == n_st - 1))
                        # ---- normalize ----
                        for ci, (c0, cw) in enumerate(pair):
                            ao_h = aoP.tile([128, CW], f32r, name="ao", tag="ao", bufs=2 * HPC + 2)
                            nc.vector.tensor_mul(ao_h[:, :cw], o2[ci][:, :cw], zr2s[ci][:, :cw])
                            ao[(h, ci)] = ao_h
                    # ---- out projection for this pair (PSUM slots reused from sc pool) ----
                    for ci, (c0, cw) in enumerate(pair):
                        for tt in range(cw // 128):
                            y_sb = yP.tile([128, DIM], f32, name="y_sb", tag="ysb", bufs=2)
                            for nn in range(DIM // 512):
                                yps = ps_sc.tile([128, 512], f32, name="yps", tag="sc", bufs=2)
                                for h in range(HPC):
                                    nc.tensor.matmul(
                                        yps[:],
                                        ao[(h, ci)][:, tt * 128:(tt + 1) * 128],
                                        wo[h][:, nn * 512:(nn + 1) * 512],
                                        start=(h == 0), stop=(h == HPC - 1))
                                nc.vector.tensor_copy(y_sb[:, nn * 512:(nn + 1) * 512], yps[:])
                            nc.sync.dma_start(out=y[c0 + tt * 128: c0 + (tt + 1) * 128, :], in_=y_sb[:])

    nc.compile()
    return nc


def _host_tables(T=T, S=S):
    scale = float(HD) ** (-0.25)
    inv = 1.0 / (ROPE_BASE ** (np.arange(0, HD, 2, dtype=np.float64) / HD))  # [64]

    def tables(L):
        fr = np.outer(inv, np.arange(L, dtype=np.float64))  # [64, L]
        c = np.cos(fr) * scale
        s = np.sin(fr) * scale
        cos = np.concatenate([c, c], axis=0).astype(np.float32)
        sin = np.concatenate([-s, s], axis=0).astype(np.float32)
        return np.ascontiguousarray(cos), np.ascontiguousarray(sin)

    cosq_, sinq_ = tables(T)
    cosk_, sink_ = tables(S)
    return cosq_, sinq_, cosk_, sink_


def make_in_maps(x, xmel, Wq, Wkv, Wout):
    import ml_dtypes
    bf = ml_dtypes.bfloat16
    Bx, Tx, C = x.shape
    Sx = xmel.shape[1]
    cosq_, sinq_, cosk_, sink_ = _host_tables(Tx, Sx)

    x = np.asarray(x, dtype=np.float32)
    xmel = np.asarray(xmel, dtype=np.float32)
    Wq = np.asarray(Wq, dtype=np.float32)
    Wkv = np.asarray(Wkv, dtype=np.float32)
    Wout = np.asarray(Wout, dtype=np.float32)

    xT_b = [np.ascontiguousarray(x[b].T).astype(bf) for b in range(Bx)]
    xmelT_b = [np.ascontiguousarray(xmel[b].T).astype(bf) for b in range(Bx)]
    gsz = HPC * HD  # 256
    WqT_g, WkT_g, WvT_g, WoT_g = [], [], [], []
    for g in range(NG):
        r0 = g * gsz
        WqT_g.append(np.ascontiguousarray(Wq[r0:r0 + gsz, :].T).astype(bf))
        WkT_g.append(np.ascontiguousarray(Wkv[r0:r0 + gsz, :].T).astype(bf))
        WvT_g.append(np.ascontiguousarray(Wkv[DIM + r0:DIM + r0 + gsz, :].T).astype(bf))
        WoT_g.append(np.ascontiguousarray(Wout[:, r0:r0 + gsz].T))

    in_maps = []
    for c in range(Bx * NG):
        b, g = c // NG, c % NG
        in_maps.append({
            "xT": xT_b[b], "xmelT": xmelT_b[b],
            "WqT": WqT_g[g], "WkT": WkT_g[g], "WvT": WvT_g[g], "WoT": WoT_g[g],
            "cosq": cosq_, "sinq": sinq_, "cosk": cosk_, "sink": sink_,
        })
    return in_maps


def kernel(x, xmel, Wq, Wkv, Wout):
    from concourse.bass_utils import run_bass_kernel_spmd

    x = np.asarray(x, dtype=np.float32)
    xmel = np.asarray(xmel, dtype=np.float32)
    Bx, Tx, C = x.shape
    Sx = xmel.shape[1]
    assert (Bx, Tx, C, Sx) == (B, T, DIM, S)

    if "nc" not in _cache:
        _cache["nc"] = build_nc()
    nc = _cache["nc"]

    in_maps = make_in_maps(x, xmel,
                           np.asarray(Wq, dtype=np.float32),
                           np.asarray(Wkv, dtype=np.float32),
                           np.asarray(Wout, dtype=np.float32))
    res = run_bass_kernel_spmd(nc, in_maps, list(range(8)))
    out = np.zeros((B, T, DIM), dtype=np.float32)
    for c in range(8):
        b = c // NG
        out[b] += res.results[c]["y"]
    return out


# revision 19
# speedup vs baseline: 1.3502x; 1.1137x over previous
"""Trainium2 Bass kernel for MHA cross-attention (nn_MHACross).

Sharding: 8 cores = 2 batches x 4 head-groups (2 heads each).
Each core computes, for its (batch b, head group g):
    q = x[b] @ Wq[g].T ; k,v = xmel[b] @ Wkv[g].T ; RoPE(q, k) (scale folded
    into host-side cos/sin tables); per head scores^T = k_r @ q_r^T;
    p = exp(scores) with no max subtraction (scores are O(6) here, safe in
    fp32); unnormalized out2 = v^T @ p and Z = ones^T @ p on the PE;
    normalize by 1/Z; y_partial = attn @ Wout[:, g].T.  Host sums the 4
    partial y per batch.

Layouts keep the contraction dim on partitions everywhere; no on-device
transposes.  Matmul operands are bf16 (except the final projection, which
runs in float32r); PSUM accumulation is fp32 throughout.  The attention
inner loop is batched by op type (all scores, then all attnV, then all Z
matmuls per head/chunk-pair) so the PE streams back-to-back with stationary
reuse, while exp for both 512-chunks of a pair runs as one [128,1024]
scalar-engine instruction.
"""
import sys
sys.path.insert(0, '/opt/trn_rl_repo')
import numpy as np

DIM = 1024
NHEADS = 8
HD = 128          # head dim
HPC = 2           # heads per core
NG = 4            # head groups (cores per batch)
B, T, S = 2, 2048, 3000
NKT = DIM // 128  # contraction tiles
ROPE_BASE = 10000.0
CW = 512          # T-chunk width
PAIR = 2 * CW     # paired chunk width for exp

_cache = {}


def _ceil_div(a, b):
    return (a + b - 1) // b


def build_nc(T=T, S=S):
    from concourse import bacc, mybir
    from concourse.tile import TileContext

    f32 = mybir.dt.float32
    f32r = mybir.dt.float32r
    bf16 = mybir.dt.bfloat16

    nc = bacc.Bacc("TRN2", target_bir_lowering=False, debug=False, num_devices=8)

    xT = nc.dram_tensor("xT", [DIM, T], bf16, kind="ExternalInput")
    xmelT = nc.dram_tensor("xmelT", [DIM, S], bf16, kind="ExternalInput")
    WqT = nc.dram_tensor("WqT", [DIM, HPC * HD], bf16, kind="ExternalInput")
    WkT = nc.dram_tensor("WkT", [DIM, HPC * HD], bf16, kind="ExternalInput")
    WvT = nc.dram_tensor("WvT", [DIM, HPC * HD], bf16, kind="ExternalInput")
    WoT = nc.dram_tensor("WoT", [HPC * HD, DIM], bf16, kind="ExternalInput")
    cosq = nc.dram_tensor("cosq", [HD, T], f32, kind="ExternalInput")
    sinq = nc.dram_tensor("sinq", [HD, T], f32, kind="ExternalInput")
    cosk = nc.dram_tensor("cosk", [HD, S], f32, kind="ExternalInput")
    sink = nc.dram_tensor("sink", [HD, S], f32, kind="ExternalInput")
    y = nc.dram_tensor("y", [T, DIM], f32, kind="ExternalOutput")
    # DRAM scratch for the Z-broadcast bounce (internal DRAM tensors fail to
    # load under the axon PJRT path, so declare it as an output)
    n_tc = _ceil_div(T, CW)
    zsd = nc.dram_tensor("zs", [HPC * n_tc, CW], f32, kind="ExternalOutput")

    n_st = _ceil_div(S, 128)
    s_chunks = [(i * 512, min(512, S - i * 512)) for i in range(_ceil_div(S, 512))]
    t_chunks = [(i * CW, min(CW, T - i * CW)) for i in range(n_tc)]
    t_pairs = [t_chunks[i:i + 2] for i in range(0, n_tc, 2)]

    with TileContext(nc) as tc:
        with tc.tile_pool(name="wpool", bufs=1) as wp, \
             tc.tile_pool(name="persist", bufs=1) as pp:
            # persistent weights
            wq = wp.tile([128, NKT, HPC * HD], bf16)
            wk = wp.tile([128, NKT, HPC * HD], bf16)
            wv = wp.tile([128, NKT, HPC * HD], bf16)
            wo = []
            for h in range(HPC):
                wo_h = wp.tile([128, DIM], bf16, name=f"wo{h}", uniquify=True)
                nc.sync.dma_start(out=wo_h[:], in_=WoT[h * HD:(h + 1) * HD, :])
                wo.append(wo_h)
            ones = wp.tile([128, 1], bf16)
            nc.vector.memset(ones[:], 1.0)

            # persistent activations
            kT_r = [pp.tile([128, S], bf16, name=f"kT{h}", uniquify=True) for h in range(HPC)]
            qT_r = [pp.tile([128, T], bf16, name=f"qT{h}", uniquify=True) for h in range(HPC)]
            v_sb = pp.tile([128, n_st, HPC * HD], bf16)

            # ------------- phase 1: projections + rope (q first) -------------
            with tc.tile_pool(name="xqp", bufs=NKT) as xqp, \
                 tc.tile_pool(name="xmelp", bufs=NKT) as xp, \
                 tc.tile_pool(name="csP", bufs=4) as csp, \
                 tc.tile_pool(name="rtP", bufs=3) as rtp, \
                 tc.tile_pool(name="ps_pr", bufs=3, space="PSUM") as pspr, \
                 tc.tile_pool(name="ps_v", bufs=2, space="PSUM") as psv:
                # x first (smaller), so q-proj compute overlaps the xmel DMA
                nc.sync.dma_start(out=wq[:], in_=WqT[:].rearrange("(k p) n -> p k n", p=128))
                nc.sync.dma_start(out=wk[:], in_=WkT[:].rearrange("(k p) n -> p k n", p=128))
                nc.sync.dma_start(out=wv[:], in_=WvT[:].rearrange("(k p) n -> p k n", p=128))
                xq = []
                for kt in range(NKT):
                    xq_t = xqp.tile([128, T], bf16, name=f"xq{kt}", uniquify=True, tag="xq", bufs=NKT)
                    nc.sync.dma_start(out=xq_t[:], in_=xT[kt * 128:(kt + 1) * 128, :])
                    xq.append(xq_t)
                xm = []
                for kt in range(NKT):
                    xm_t = xp.tile([128, S], bf16, name=f"xm{kt}", uniquify=True, tag="xm", bufs=NKT)
                    nc.sync.dma_start(out=xm_t[:], in_=xmelT[kt * 128:(kt + 1) * 128, :])
                    xm.append(xm_t)

                def proj_rope(h, c0, cw, w_sb, src, cos_sb, sin_sb, out_sl):
                    ps = pspr.tile([128, 512], f32, name="prps", tag="prps", bufs=3)
                    for kt in range(NKT):
                        nc.tensor.matmul(
                            ps[:, :cw],
                            w_sb[:, kt, h * HD:(h + 1) * HD],
                            src[kt][:, c0:c0 + cw],
                            start=(kt == 0), stop=(kt == NKT - 1))
                    swp = rtp.tile([128, 512], f32, name="swp", tag="rt", bufs=3)
                    nc.scalar.copy(swp[0:64, :cw], ps[64:128, :cw])
                    nc.scalar.copy(swp[64:128, :cw], ps[0:64, :cw])
                    nc.vector.tensor_mul(swp[:, :cw], swp[:, :cw], sin_sb[:, :cw])
                    nc.vector.tensor_mul(out_sl, ps[:, :cw], cos_sb[:, :cw])
                    nc.vector.tensor_add(out_sl, out_sl, swp[:, :cw])

                def load_cs(cos_d, sin_d, c0, cw):
                    cos_sb = csp.tile([128, 512], f32, name="cos_sb", tag="cos", bufs=2)
                    sin_sb = csp.tile([128, 512], f32, name="sin_sb", tag="sin", bufs=2)
                    nc.gpsimd.dma_start(out=cos_sb[:, :cw], in_=cos_d[:, c0:c0 + cw])
                    nc.gpsimd.dma_start(out=sin_sb[:, :cw], in_=sin_d[:, c0:c0 + cw])
                    return cos_sb, sin_sb

                for (c0, cw) in t_chunks:
                    cos_sb, sin_sb = load_cs(cosq, sinq, c0, cw)
                    for h in range(HPC):
                        proj_rope(h, c0, cw, wq, xq, cos_sb, sin_sb, qT_r[h][:, c0:c0 + cw])
                for (c0, cw) in s_chunks:
                    cos_sb, sin_sb = load_cs(cosk, sink, c0, cw)
                    for h in range(HPC):
                        proj_rope(h, c0, cw, wk, xm, cos_sb, sin_sb, kT_r[h][:, c0:c0 + cw])
                for st in range(n_st):
                    s0 = st * 128
                    scnt = min(128, S - s0)
                    vps = psv.tile([128, HPC * HD], f32, name="vps", tag="vps", bufs=2)
                    for kt in range(NKT):
                        nc.tensor.matmul(
                            vps[:scnt, :],
                            xm[kt][:, s0:s0 + scnt],
                            wv[:, kt, :],
                            start=(kt == 0), stop=(kt == NKT - 1))
                    nc.vector.tensor_copy(v_sb[:scnt, st, :], vps[:scnt, :])

            # ------------- phase 2: attention + out projection -------------
            with tc.tile_pool(name="pP", bufs=n_st + 2) as pP, \
                 tc.tile_pool(name="aoP", bufs=2 * HPC + 2) as aoP, \
                 tc.tile_pool(name="zP", bufs=4) as zP, \
                 tc.tile_pool(name="yP", bufs=2) as yP, \
                 tc.tile_pool(name="ps_sc", bufs=2, space="PSUM") as ps_sc, \
                 tc.tile_pool(name="ps_o2", bufs=2, space="PSUM") as ps_o2, \
                 tc.tile_pool(name="ps_z", bufs=2, space="PSUM") as ps_z:
                for pair in t_pairs:
                    pw = sum(cw for _, cw in pair)       # total pair width
                    p0 = pair[0][0]                      # pair base column
                    ao = {}
                    for h in range(HPC):
                        # ---- scores + exp, batched over S-tiles ----
                        ptiles = []
                        for st in range(n_st):
                            s0 = st * 128
                            scnt = min(128, S - s0)
                            scps = ps_sc.tile([128, PAIR], f32, name="scps", tag="sc", bufs=2)
                            for ci, (c0, cw) in enumerate(pair):
                                nc.tensor.matmul(
                                    scps[:scnt, ci * CW: ci * CW + cw],
                                    kT_r[h][:, s0:s0 + scnt],
                                    qT_r[h][:, c0:c0 + cw],
                                    start=True, stop=True,
                                    skip_group_check=True)
                            p_t = pP.tile([128, PAIR], bf16, name="p_t", tag="p", bufs=n_st + 2)
                            nc.scalar.activation(p_t[:scnt, :pw], scps[:scnt, :pw],
                                                 mybir.ActivationFunctionType.Exp)
                            ptiles.append((p_t, scnt))
                        # ---- Z first, batched (ones stationary loaded once) ----
                        zps = []
                        for ci, (c0, cw) in enumerate(pair):
                            z_ps = ps_z.tile([1, CW], f32, name="zps", tag="z", bufs=2)
                            zps.append(z_ps)
                        for st in range(n_st):
                            p_t, scnt = ptiles[st]
                            for ci, (c0, cw) in enumerate(pair):
                                nc.tensor.matmul(
                                    zps[ci][:, :cw],
                                    ones[:scnt, :],
                                    p_t[:scnt, ci * CW: ci * CW + cw],
                                    start=(st == 0), stop=(st == n_st - 1))
                        # 1/Z computed while the attnV matmuls stream below
                        zr2s = []
                        for ci, (c0, cw) in enumerate(pair):
                            zrow = h * n_tc + (c0 // CW)
                            zsb = zP.tile([1, CW], f32, name="zsb", tag="zsb", bufs=2)
                            nc.vector.tensor_copy(zsb[:, :cw], zps[ci][:, :cw])
                            nc.sync.dma_start(out=zsd[zrow:zrow + 1, :cw], in_=zsb[:, :cw])
                            zrep = zP.tile([128, CW], f32, name="zrep", tag="zrep", bufs=2)
                            nc.sync.dma_start(out=zrep[:, :cw], in_=zsd[zrow, :cw].partition_broadcast(128))
                            zr2 = zP.tile([128, CW], f32, name="zr2", tag="zr2", bufs=2)
                            nc.vector.reciprocal_approx_fast(out=zr2[:, :cw], in_=zrep[:, :cw])
                            zr2s.append(zr2)
                        # ---- attnV, batched (v stationary amortized) ----
                        o2 = []
                        for ci, (c0, cw) in enumerate(pair):
                            o2ps = ps_o2.tile([128, CW], f32, name="o2ps", tag="o2", bufs=2)
                            o2.append(o2ps)
                        for st in range(n_st):
                            p_t, scnt = ptiles[st]
                            for ci, (c0, cw) in enumerate(pair):
                                nc.tensor.matmul(
                                    o2[ci][:, :cw],
                                    v_sb[:scnt, st, h * HD:(h + 1) * HD],
                                    p_t[:scnt, ci * CW: ci * CW + cw],
                                    start=(st == 0), stop=(st == n_st - 1))
                        # ---- normalize ----
                        for ci, (c0, cw) in enumerate(pair):
                            ao_h = aoP.tile([128, CW], bf16, name="ao", tag="ao", bufs=2 * HPC + 2)
                            nc.vector.tensor_mul(ao_h[:, :cw], o2[ci][:, :cw], zr2s[ci][:, :cw])
                            ao[(h, ci)] = ao_h
                    # ---- out projection for this pair (PSUM slots reused from sc pool) ----
                    for ci, (c0, cw) in enumerate(pair):
                        for tt in range(cw // 128):
                            y_sb = yP.tile([128, DIM], f32, name="y_sb", tag="ysb", bufs=2)
                            for nn in range(DIM // 512):
                                yps = ps_sc.tile([128, 512], f32, name="yps", tag="sc", bufs=2)
                                for h in range(HPC):
                                    nc.tensor.matmul(
                                        yps[:],
                                        ao[(h, ci)][:, tt * 128:(tt + 1) * 128],
                                        wo[h][:, nn * 512:(nn + 1) * 512],
                                        start=(h == 0), stop=(h == HPC - 1))
                                nc.vector.tensor_copy(y_sb[:, nn * 512:(nn + 1) * 512], yps[:])
                            nc.sync.dma_start(out=y[c0 + tt * 128: c0 + (tt + 1) * 128, :], in_=y_sb[:])

    nc.compile()
    return nc


def _host_tables(T=T, S=S):
    scale = float(HD) ** (-0.25)
    inv = 1.0 / (ROPE_BASE ** (np.arange(0, HD, 2, dtype=np.float64) / HD))  # [64]

    def tables(L):
        fr = np.outer(inv, np.arange(L, dtype=np.float64))  # [64, L]
        c = np.cos(fr) * scale
        s = np.sin(fr) * scale
        cos = np.concatenate([c, c], axis=0).astype(np.float32)
        sin = np.concatenate([-s, s], axis=0).astype(np.float32)
        return np.ascontiguousarray(cos), np.ascontiguousarray(sin)

    cosq_, sinq_ = tables(T)
    cosk_, sink_ = tables(S)
    return cosq_, sinq_, cosk_, sink_


def make_in_maps(x, xmel, Wq, Wkv, Wout):
    import ml_dtypes
    bf = ml_dtypes.bfloat16
    Bx, Tx, C = x.shape
    Sx = xmel.shape[1]
    cosq_, sinq_, cosk_, sink_ = _host_tables(Tx, Sx)

    x = np.asarray(x, dtype=np.float32)
    xmel = np.asarray(xmel, dtype=np.float32)
    Wq = np.asarray(Wq, dtype=np.float32)
    Wkv = np.asarray(Wkv, dtype=np.float32)
    Wout = np.asarray(Wout, dtype=np.float32)

    xT_b = [np.ascontiguousarray(x[b].T).astype(bf) for b in range(Bx)]
    xmelT_b = [np.ascontiguousarray(xmel[b].T).astype(bf) for b in range(Bx)]
    gsz = HPC * HD  # 256
    WqT_g, WkT_g, WvT_g, WoT_g = [], [], [], []
    for g in range(NG):
        r0 = g * gsz
        WqT_g.append(np.ascontiguousarray(Wq[r0:r0 + gsz, :].T).astype(bf))
        WkT_g.append(np.ascontiguousarray(Wkv[r0:r0 + gsz, :].T).astype(bf))
        WvT_g.append(np.ascontiguousarray(Wkv[DIM + r0:DIM + r0 + gsz, :].T).astype(bf))
        WoT_g.append(np.ascontiguousarray(Wout[:, r0:r0 + gsz].T).astype(bf))

    in_maps = []
    for c in range(Bx * NG):
        b, g = c // NG, c % NG
        in_maps.append({
            "xT": xT_b[b], "xmelT": xmelT_b[b],
            "WqT": WqT_g[g], "WkT": WkT_g[g], "WvT": WvT_g[g], "WoT": WoT_g[g],
            "cosq": cosq_, "sinq": sinq_, "cosk": cosk_, "sink": sink_,
        })
    return in_maps


def kernel(x, xmel, Wq, Wkv, Wout):
    from concourse.bass_utils import run_bass_kernel_spmd

    x = np.asarray(x, dtype=np.float32)
    xmel = np.asarray(xmel, dtype=np.float32)
    Bx, Tx, C = x.shape
    Sx = xmel.shape[1]
    assert (Bx, Tx, C, Sx) == (B, T, DIM, S)

    if "nc" not in _cache:
        _cache["nc"] = build_nc()
    nc = _cache["nc"]

    in_maps = make_in_maps(x, xmel,
                           np.asarray(Wq, dtype=np.float32),
                           np.asarray(Wkv, dtype=np.float32),
                           np.asarray(Wout, dtype=np.float32))
    res = run_bass_kernel_spmd(nc, in_maps, list(range(8)))
    out = np.zeros((B, T, DIM), dtype=np.float32)
    for c in range(8):
        b = c // NG
        out[b] += res.results[c]["y"]
    return out
